# revision 19
# baseline (speedup 1.0000x reference)
"""TopK SAE (encode -> top-32 mask -> decode) on 8 trn2 NeuronCores.

Data-parallel over the batch dim N: each core handles N/8 = 512 rows.
W_enc is pre-transposed on the host (the PE contracts over the partition
dim, so the encode needs W_enc^T with D on partitions); W_dec is host-cast
to fp16 for the decode (products accumulate in fp32 in PSUM).

Per-core device pipeline (single pass over 4 row-tiles of 128):
  encode:   raw = x_cent @ W_enc^T via float32r (FP22) matmuls, full PE
            rate, one stream of W_enc^T from HBM
  stage1:   per 512-wide f-tile: PSUM -> fp32 staging chunk, nc.vector.max
            top-8 -> 256 exact candidates/row; chunk cast to a fp16
            resident raw (the only full-row copy SBUF can hold)
  stage2:   6x (max8 + match_replace) over the candidates -> exact
            thresholds: rank-32 (acts mask) and rank-48 (host audit mask)
  mask:     acts16 = (raw16 >= t32) * raw16 fused in place; mask48 =
            (raw16 >= t48) u8 shipped out for the host-side exact audit
  decode:   PE-transpose acts16 128x128 blocks, copy back via ScalarE,
            accumulate recon = acts @ W_dec over 128 f-chunks
Host: shard/transpose/cast prep, gather, exact re-ranking of each row's
top-48 boundary (fixes FP22/fp16 winner swaps on a few dozen rows), loss.
"""

import os
import sys

for _p in ("/opt/trn_rl_repo", "/opt/pypackages"):
    if _p not in sys.path:
        sys.path.insert(0, _p)

import numpy as np

import concourse.bacc as bacc
import concourse.mybir as mybir
from concourse.bass import ds, ts
from concourse.masks import make_identity
from concourse.tile import TileContext
from concourse.bass_utils import run_bass_kernel_spmd

# Problem shape (hardcoded per contract)
N, D, F, K = 4096, 512, 16384, 32
NCORES = 8
NS = N // NCORES          # rows per core = 512
P = 128
NT = NS // P              # 4 row-tiles per core
FT = 512                  # encode f-tile width (= stage1 chunk width)
NFT = F // FT             # 32
FG = 2                    # f-tiles per W-load group
NCAND = NFT * 8           # 256 candidates per row
FC = 128                  # decode f-chunk (transpose block)
NFC = F // FC             # 128
DECG = 4                  # decode f-chunks per group (one PSUM bank)
MCH = 2048                # mask/acts chunk width
NEG = -3.0e38

f32 = mybir.dt.float32
f32r = mybir.dt.float32r
f16 = mybir.dt.float16
u8 = mybir.dt.uint8


def build(dca: int):
    """Build the per-core Bass program. dca = number of 128-deep contraction
    chunks (4 normally, 5 when b_enc is folded in as an extra chunk)."""
    nc = bacc.Bacc("TRN2", target_bir_lowering=False)

    xT = nc.dram_tensor("xT", [dca * P, NS], f16, kind="ExternalInput")
    wencT = nc.dram_tensor("wencT", [dca * P, F], f16, kind="ExternalInput")
    wdec16 = nc.dram_tensor("wdec16", [F, D], f16, kind="ExternalInput")
    acts_d = nc.dram_tensor("acts", [NS, F], f16, kind="ExternalOutput")
    recon_d = nc.dram_tensor("recon", [NS, D], f32, kind="ExternalOutput")
    mask48_d = nc.dram_tensor("mask48", [NS, F], u8, kind="ExternalOutput")

    xT_r = xT.rearrange("(dc p) n -> p dc n", p=P)
    wencT_r = wencT.rearrange("(dc p) f -> p dc f", p=P)
    wdec_r = wdec16.rearrange("(c p) d -> p c d", p=P)

    with TileContext(nc) as tc:
        with (
            tc.tile_pool(name="persist", bufs=1) as persist,
            tc.tile_pool(name="wt", bufs=2) as wtpool,
            tc.tile_pool(name="raw", bufs=NT) as rawpool,
            tc.tile_pool(name="stage", bufs=2) as stagepool,
            tc.tile_pool(name="cand", bufs=NT) as candpool,
            tc.tile_pool(name="work", bufs=1) as workpool,
            tc.tile_pool(name="m8", bufs=NT) as m8pool,
            tc.tile_pool(name="at", bufs=2) as atpool,
            tc.tile_pool(name="wd", bufs=2) as wdpool,
            tc.tile_pool(name="rc", bufs=1) as rcpool,
            tc.tile_pool(name="m48", bufs=1) as m48pool,
            tc.tile_pool(name="encps", bufs=4, space="PSUM") as encps,
            tc.tile_pool(name="trps", bufs=2, space="PSUM") as trps,
            tc.tile_pool(name="decps", bufs=2, space="PSUM") as decps,
        ):
            ident = persist.tile([P, P], f16)
            make_identity(nc, ident)

            xt_sb = persist.tile([P, dca, NS], f16)
            nc.sync.dma_start(xt_sb[:], xT_r[:])

            raws = []
            cands = []
            for nt in range(NT):
                raws.append(rawpool.tile([P, F], f16, tag="raw", name=f"raw{nt}"))
                cands.append(
                    candpool.tile([P, NCAND], f32, tag="cand", name=f"cand{nt}")
                )

            # ---- encode + stage1 candidates (per half of row-tiles) ----
            def encode_half(half):
                nts = [half * 2, half * 2 + 1]
                for fg in range(NFT // FG):
                    wt = wtpool.tile([P, dca, FG * FT], f16, tag="wt")
                    for dc in range(dca):
                        nc.sync.dma_start(
                            wt[:, dc, :],
                            wencT_r[:, dc, ds(fg * FG * FT, FG * FT)],
                        )
                    for nt in nts:
                        pss = [
                            encps.tile(
                                [P, FT], f32, tag="encps", name=f"encps{i}"
                            )
                            for i in range(FG)
                        ]
                        for dc in range(dca):
                            for ft in range(FG):
                                nc.tensor.matmul(
                                    pss[ft],
                                    lhsT=xt_sb[:, dc, ds(nt * P, P)],
                                    rhs=wt[:, dc, ds(ft * FT, FT)],
                                    start=(dc == 0),
                                    stop=(dc == dca - 1),
                                )
                        for ft in range(FG):
                            fa = fg * FG + ft
                            st = stagepool.tile([P, FT], f32, tag="stage")
                            nc.scalar.copy(st[:], pss[ft])
                            nc.vector.max(
                                out=cands[nt][:, ds(fa * 8, 8)], in_=st[:]
                            )
                            nc.vector.tensor_copy(
                                raws[nt][:, ds(fa * FT, FT)], st[:]
                            )

            # ---- stage2: exact thresholds from candidates ----
            # 6 max8 rounds: rank K=32 -> acts threshold, rank 48 -> slack
            # threshold whose u8 mask lets the host re-rank the boundary.
            t_aps = [None] * NT
            t48_aps = [None] * NT

            def stage2_nt(nt):
                work = workpool.tile([P, NCAND], f32, tag="work")
                nc.vector.tensor_copy(work[:], cands[nt][:])
                niter = 6
                m8 = m8pool.tile([P, niter * 8], f32, tag="m8", name=f"m8_{nt}")
                for j in range(niter):
                    nc.vector.max(out=m8[:, ds(j * 8, 8)], in_=work[:])
                    if j < niter - 1:
                        nc.vector.match_replace(
                            out=work[:],
                            in_to_replace=m8[:, ds(j * 8, 8)],
                            in_values=work[:],
                            imm_value=NEG,
                        )
                t_aps[nt] = m8[:, ds(K - 1, 1)]
                t48_aps[nt] = m8[:, ds(niter * 8 - 1, 1)]

            # ---- mask + decode, interleaved in two halves so the DVE
            # mask work of half 2 overlaps the PE decode of half 1 ----
            def mask_nt(nt):
                for ch in range(F // MCH):
                    sl = ds(ch * MCH, MCH)
                    m48 = m48pool.tile([P, MCH // 2], u8, tag="m48")
                    for h in range(2):
                        slh = ds(ch * MCH + h * (MCH // 2), MCH // 2)
                        nc.vector.tensor_scalar(
                            m48[:],
                            raws[nt][:, slh],
                            t48_aps[nt],
                            None,
                            op0=mybir.AluOpType.is_ge,
                        )
                        nc.sync.dma_start(mask48_d[ds(nt * P, P), slh], m48[:])
                    nc.vector.scalar_tensor_tensor(
                        out=raws[nt][:, sl],
                        in0=raws[nt][:, sl],
                        scalar=t_aps[nt],
                        in1=raws[nt][:, sl],
                        op0=mybir.AluOpType.is_ge,
                        op1=mybir.AluOpType.mult,
                    )
                    nc.sync.dma_start(
                        acts_d[ds(nt * P, P), sl], raws[nt][:, sl]
                    )

            def decode_half(half):
                nts = [half * 2, half * 2 + 1]
                dec_ps = {
                    nt: decps.tile([P, D], f32, tag="decps", name=f"decps{nt}")
                    for nt in nts
                }
                for g in range(NFC // DECG):
                    wd = wdpool.tile([P, DECG, D], f16, tag="wd")
                    nc.sync.dma_start(wd[:], wdec_r[:, ds(g * DECG, DECG), :])
                    for nt in nts:
                        at = atpool.tile([P, DECG, P], f16, tag="at")
                        for j in range(DECG):
                            nc.scalar.dma_start_transpose(
                                at[:, j, :],
                                raws[nt][:, ds((g * DECG + j) * P, P)],
                            )
                        for j in range(DECG):
                            nc.tensor.matmul(
                                dec_ps[nt],
                                lhsT=at[:, j, :],
                                rhs=wd[:, j, :],
                                start=(g == 0 and j == 0),
                                stop=(g == NFC // DECG - 1 and j == DECG - 1),
                            )
                for nt in nts:
                    rc = rcpool.tile([P, D], f32, tag="rc")
                    nc.scalar.copy(rc[:], dec_ps[nt])
                    nc.sync.dma_start(recon_d[ds(nt * P, P), :], rc[:])

            encode_half(0)
            stage2_nt(0)
            stage2_nt(1)
            mask_nt(0)
            mask_nt(1)
            encode_half(1)
            stage2_nt(2)
            stage2_nt(3)
            mask_nt(2)
            mask_nt(3)
            decode_half(0)
            decode_half(1)

    nc.compile()
    return nc


_cache = {}


def _get_nc(dca: int):
    if dca not in _cache:
        _cache[dca] = build(dca)
    return _cache[dca]


def run(inputs, trace=False, trace_cores=None):
    x = np.asarray(inputs["x"], dtype=np.float32)
    W_enc = np.asarray(inputs["W_enc"], dtype=np.float32)
    W_dec = np.asarray(inputs["W_dec"], dtype=np.float32)
    b_enc = np.asarray(inputs["b_enc"], dtype=np.float32)
    b_dec = np.asarray(inputs["b_dec"], dtype=np.float32)
    k = int(np.asarray(inputs["num_winners"]))
    assert k == K, f"kernel specialized for K={K}, got {k}"
    assert x.shape == (N, D) and W_enc.shape == (F, D)

    x_cent = x - b_dec[None, :]

    has_benc = bool(np.any(b_enc))
    dca = D // P + (1 if has_benc else 0)

    # host-side weight prep (layout for the PE): W_enc^T with D on
    # partitions; optional extra contraction chunk folds b_enc in via an
    # all-ones row of x.
    wencT = np.ascontiguousarray(W_enc.T).astype(np.float16)  # [D, F]
    if has_benc:
        pad = np.zeros((P, F), np.float16)
        pad[0, :] = b_enc.astype(np.float16)
        wencT = np.concatenate([wencT, pad], axis=0)
    wdec16 = W_dec.astype(np.float16)              # [F, D]

    nc = _get_nc(dca)

    in_maps = []
    for c in range(NCORES):
        xs = x_cent[c * NS : (c + 1) * NS]          # [NS, D]
        xsT = np.ascontiguousarray(xs.T).astype(np.float16)  # [D, NS]
        if has_benc:
            pad = np.zeros((P, NS), np.float16)
            pad[0, :] = 1.0
            xsT = np.concatenate([xsT, pad], axis=0)
        in_maps.append({"xT": xsT, "wencT": wencT, "wdec16": wdec16})

    res = run_bass_kernel_spmd(
        nc,
        in_maps,
        core_ids=list(range(NCORES)),
        trace=trace,
        trace_cores=trace_cores,
    )

    acts = np.concatenate(
        [res.results[c]["acts"] for c in range(NCORES)], axis=0
    ).astype(np.float32)
    recon = np.concatenate(
        [res.results[c]["recon"] for c in range(NCORES)], axis=0
    )
    mask48 = np.concatenate(
        [res.results[c]["mask48"] for c in range(NCORES)], axis=0
    )
    recon = recon + b_dec[None, :]

    # Exact boundary re-ranking. The device encode runs at FP22 (float32r)
    # and the resident raw copy is fp16, so winners whose fp32 gap is tiny
    # can swap or carry rounded values. mask48 marks each row's top-~48
    # device values; recompute those dot products exactly in fp32, rewrite
    # the row's true top-32 entries, and re-decode the few changed rows.
    rows, cols = np.nonzero(mask48)
    row_starts = np.searchsorted(rows, np.arange(N + 1))
    exact = np.einsum(
        "nd,nd->n", x_cent[rows], W_enc[cols], dtype=np.float32
    ) + b_enc[cols]
    for n in range(N):
        s, e = row_starts[n], row_starts[n + 1]
        cs = cols[s:e]
        ev = exact[s:e]
        if len(cs) < K:
            continue
        order = np.argsort(-ev, kind="stable")
        true_set = cs[order[:K]]
        sel = cs[acts[n, cs] != 0]
        changed = not (
            len(sel) == K and np.array_equal(np.sort(true_set), np.sort(sel))
        )
        acts[n, sel] = 0.0
        acts[n, true_set] = ev[order[:K]]
        if changed:
            recon[n] = acts[n, true_set] @ W_dec[true_set] + b_dec

    diff = recon.astype(np.float32) - x
    loss = np.float32(np.mean(np.sum(diff * diff, axis=-1, dtype=np.float32)))
    return (loss, recon, acts), res


def kernel(**inputs):
    out, _ = run(inputs, trace=False)
    return out


# revision 21
# speedup vs baseline: 3.3269x; 3.3269x over previous
"""TopK SAE (encode -> top-32 mask -> decode) on 8 trn2 NeuronCores.

Data-parallel over the batch dim N: each core handles N/8 = 512 rows.
W_enc is pre-transposed on the host (the PE contracts over the partition
dim, so the encode needs W_enc^T with D on partitions); W_dec is host-cast
to fp16 for the decode (products accumulate in fp32 in PSUM).

Per-core device pipeline (single pass over 4 row-tiles of 128):
  encode:   raw = x_cent @ W_enc^T via float32r (FP22) matmuls, full PE
            rate, one stream of W_enc^T from HBM
  stage1:   per 512-wide f-tile: PSUM -> fp32 staging chunk, nc.vector.max
            top-8 -> 256 exact candidates/row; chunk cast to a fp16
            resident raw (the only full-row copy SBUF can hold)
  stage2:   6x (max8 + match_replace) over the candidates -> exact
            thresholds: rank-32 (acts mask) and rank-48 (host audit mask)
  mask:     acts16 = (raw16 >= t32) * raw16 fused in place; mask48 =
            (raw16 >= t48) u8 shipped out for the host-side exact audit
  decode:   PE-transpose acts16 128x128 blocks, copy back via ScalarE,
            accumulate recon = acts @ W_dec over 128 f-chunks
Host: shard/transpose/cast prep, gather, exact re-ranking of each row's
top-48 boundary (fixes FP22/fp16 winner swaps on a few dozen rows), loss.
"""

import os
import sys

for _p in ("/opt/trn_rl_repo", "/opt/pypackages"):
    if _p not in sys.path:
        sys.path.insert(0, _p)

import numpy as np

import concourse.bacc as bacc
import concourse.mybir as mybir
from concourse.bass import ds, ts
from concourse.masks import make_identity
from concourse.tile import TileContext
from concourse.bass_utils import run_bass_kernel_spmd

# Problem shape (hardcoded per contract)
N, D, F, K = 4096, 512, 16384, 32
NCORES = 8
NS = N // NCORES          # rows per core = 512
P = 128
NT = NS // P              # 4 row-tiles per core
FT = 512                  # encode f-tile width (= stage1 chunk width)
NFT = F // FT             # 32
FG = 2                    # f-tiles per W-load group
NCAND = NFT * 8           # 256 candidates per row
FC = 128                  # decode f-chunk (transpose block)
NFC = F // FC             # 128
DECG = 8                  # decode f-chunks per group (one PSUM bank)
MCH = 2048                # mask/acts chunk width
NEG = -3.0e38

f32 = mybir.dt.float32
f32r = mybir.dt.float32r
f16 = mybir.dt.float16
u8 = mybir.dt.uint8


def build(dca: int):
    """Build the per-core Bass program. dca = number of 128-deep contraction
    chunks (4 normally, 5 when b_enc is folded in as an extra chunk)."""
    nc = bacc.Bacc("TRN2", target_bir_lowering=False)

    xT = nc.dram_tensor("xT", [dca * P, NS], f16, kind="ExternalInput")
    wencT = nc.dram_tensor("wencT", [dca * P, F], f16, kind="ExternalInput")
    wdec16 = nc.dram_tensor("wdec16", [F, D], f16, kind="ExternalInput")
    acts_d = nc.dram_tensor("acts", [NS, F], f16, kind="ExternalOutput")
    recon_d = nc.dram_tensor("recon", [NS, D], f32, kind="ExternalOutput")
    mask48_d = nc.dram_tensor("mask48", [NS, F], u8, kind="ExternalOutput")

    xT_r = xT.rearrange("(dc p) n -> p dc n", p=P)
    wencT_r = wencT.rearrange("(dc p) f -> p dc f", p=P)
    wdec_r = wdec16.rearrange("(c p) d -> p c d", p=P)

    with TileContext(nc) as tc:
        with (
            tc.tile_pool(name="persist", bufs=1) as persist,
            tc.tile_pool(name="wt", bufs=2) as wtpool,
            tc.tile_pool(name="raw", bufs=NT) as rawpool,
            tc.tile_pool(name="stage", bufs=2) as stagepool,
            tc.tile_pool(name="cand", bufs=NT) as candpool,
            tc.tile_pool(name="work", bufs=1) as workpool,
            tc.tile_pool(name="m8", bufs=NT) as m8pool,
            tc.tile_pool(name="at", bufs=2) as atpool,
            tc.tile_pool(name="wd", bufs=2) as wdpool,
            tc.tile_pool(name="rc", bufs=1) as rcpool,
            tc.tile_pool(name="m48", bufs=2) as m48pool,
            tc.tile_pool(name="encps", bufs=4, space="PSUM") as encps,
            tc.tile_pool(name="trps", bufs=2, space="PSUM") as trps,
            tc.tile_pool(name="decps", bufs=2, space="PSUM") as decps,
        ):
            ident = persist.tile([P, P], f16)
            make_identity(nc, ident)

            xt_sb = persist.tile([P, dca, NS], f16)
            nc.sync.dma_start(xt_sb[:], xT_r[:])

            raws = []
            cands = []
            for nt in range(NT):
                raws.append(rawpool.tile([P, F], f16, tag="raw", name=f"raw{nt}"))
                cands.append(
                    candpool.tile([P, NCAND], f32, tag="cand", name=f"cand{nt}")
                )

            # ---- encode + stage1 candidates (per half of row-tiles) ----
            def encode_half(half):
                nts = [half * 2, half * 2 + 1]
                for fg in range(NFT // FG):
                    wt = wtpool.tile([P, dca, FG * FT], f16, tag="wt")
                    for dc in range(dca):
                        nc.sync.dma_start(
                            wt[:, dc, :],
                            wencT_r[:, dc, ds(fg * FG * FT, FG * FT)],
                        )
                    for nt in nts:
                        pss = [
                            encps.tile(
                                [P, FT], f32, tag="encps", name=f"encps{i}"
                            )
                            for i in range(FG)
                        ]
                        for dc in range(dca):
                            for ft in range(FG):
                                nc.tensor.matmul(
                                    pss[ft],
                                    lhsT=xt_sb[:, dc, ds(nt * P, P)],
                                    rhs=wt[:, dc, ds(ft * FT, FT)],
                                    start=(dc == 0),
                                    stop=(dc == dca - 1),
                                )
                        for ft in range(FG):
                            fa = fg * FG + ft
                            st = stagepool.tile([P, FT], f32, tag="stage")
                            nc.scalar.copy(st[:], pss[ft])
                            nc.vector.max(
                                out=cands[nt][:, ds(fa * 8, 8)], in_=st[:]
                            )
                            if ft % 2 == 0:
                                nc.scalar.copy(
                                    raws[nt][:, ds(fa * FT, FT)], pss[ft]
                                )
                            else:
                                nc.vector.tensor_copy(
                                    raws[nt][:, ds(fa * FT, FT)], st[:]
                                )

            # ---- stage2: exact thresholds from candidates ----
            # 6 max8 rounds: rank K=32 -> acts threshold, rank 48 -> slack
            # threshold whose u8 mask lets the host re-rank the boundary.
            t_aps = [None] * NT
            t48_aps = [None] * NT

            def stage2_nt(nt):
                work = workpool.tile([P, NCAND], f32, tag="work")
                nc.vector.tensor_copy(work[:], cands[nt][:])
                niter = 6
                m8 = m8pool.tile([P, niter * 8], f32, tag="m8", name=f"m8_{nt}")
                for j in range(niter):
                    nc.vector.max(out=m8[:, ds(j * 8, 8)], in_=work[:])
                    if j < niter - 1:
                        nc.vector.match_replace(
                            out=work[:],
                            in_to_replace=m8[:, ds(j * 8, 8)],
                            in_values=work[:],
                            imm_value=NEG,
                        )
                t_aps[nt] = m8[:, ds(K - 1, 1)]
                t48_aps[nt] = m8[:, ds(niter * 8 - 1, 1)]

            # ---- mask + decode, interleaved in two halves so the DVE
            # mask work of half 2 overlaps the PE decode of half 1 ----
            def mask_nt(nt):
                for ch in range(F // MCH):
                    sl = ds(ch * MCH, MCH)
                    m48 = m48pool.tile([P, MCH // 2], u8, tag="m48")
                    for h in range(2):
                        slh = ds(ch * MCH + h * (MCH // 2), MCH // 2)
                        nc.vector.tensor_scalar(
                            m48[:],
                            raws[nt][:, slh],
                            t48_aps[nt],
                            None,
                            op0=mybir.AluOpType.is_ge,
                        )
                        nc.sync.dma_start(mask48_d[ds(nt * P, P), slh], m48[:])
                    nc.vector.scalar_tensor_tensor(
                        out=raws[nt][:, sl],
                        in0=raws[nt][:, sl],
                        scalar=t_aps[nt],
                        in1=raws[nt][:, sl],
                        op0=mybir.AluOpType.is_ge,
                        op1=mybir.AluOpType.mult,
                    )
                    nc.sync.dma_start(
                        acts_d[ds(nt * P, P), sl], raws[nt][:, sl]
                    )

            def decode_half(half):
                nts = [half * 2, half * 2 + 1]
                dec_ps = {
                    nt: decps.tile([P, D], f32, tag="decps", name=f"decps{nt}")
                    for nt in nts
                }
                for g in range(NFC // DECG):
                    wd = wdpool.tile([P, DECG, D], f16, tag="wd")
                    nc.sync.dma_start(wd[:], wdec_r[:, ds(g * DECG, DECG), :])
                    for nt in nts:
                        tr = trps.tile([P, DECG, P], f16, tag="trps")
                        for j in range(DECG):
                            nc.tensor.transpose(
                                tr[:, j, :],
                                raws[nt][:, ds((g * DECG + j) * P, P)],
                                ident[:],
                            )
                        at = atpool.tile([P, DECG, P], f16, tag="at")
                        nc.scalar.copy(at[:], tr[:])
                        for j in range(DECG):
                            nc.tensor.matmul(
                                dec_ps[nt],
                                lhsT=at[:, j, :],
                                rhs=wd[:, j, :],
                                start=(g == 0 and j == 0),
                                stop=(g == NFC // DECG - 1 and j == DECG - 1),
                            )
                for nt in nts:
                    rc = rcpool.tile([P, D], f32, tag="rc")
                    nc.scalar.copy(rc[:], dec_ps[nt])
                    nc.sync.dma_start(recon_d[ds(nt * P, P), :], rc[:])

            encode_half(0)
            stage2_nt(0)
            stage2_nt(1)
            mask_nt(0)
            mask_nt(1)
            encode_half(1)
            stage2_nt(2)
            stage2_nt(3)
            mask_nt(2)
            mask_nt(3)
            decode_half(0)
            decode_half(1)

    nc.compile()
    return nc


_cache = {}


def _get_nc(dca: int):
    if dca not in _cache:
        _cache[dca] = build(dca)
    return _cache[dca]


def run(inputs, trace=False, trace_cores=None):
    x = np.asarray(inputs["x"], dtype=np.float32)
    W_enc = np.asarray(inputs["W_enc"], dtype=np.float32)
    W_dec = np.asarray(inputs["W_dec"], dtype=np.float32)
    b_enc = np.asarray(inputs["b_enc"], dtype=np.float32)
    b_dec = np.asarray(inputs["b_dec"], dtype=np.float32)
    k = int(np.asarray(inputs["num_winners"]))
    assert k == K, f"kernel specialized for K={K}, got {k}"
    assert x.shape == (N, D) and W_enc.shape == (F, D)

    x_cent = x - b_dec[None, :]

    has_benc = bool(np.any(b_enc))
    dca = D // P + (1 if has_benc else 0)

    # host-side weight prep (layout for the PE): W_enc^T with D on
    # partitions; optional extra contraction chunk folds b_enc in via an
    # all-ones row of x.
    wencT = np.ascontiguousarray(W_enc.T).astype(np.float16)  # [D, F]
    if has_benc:
        pad = np.zeros((P, F), np.float16)
        pad[0, :] = b_enc.astype(np.float16)
        wencT = np.concatenate([wencT, pad], axis=0)
    wdec16 = W_dec.astype(np.float16)              # [F, D]

    nc = _get_nc(dca)

    in_maps = []
    for c in range(NCORES):
        xs = x_cent[c * NS : (c + 1) * NS]          # [NS, D]
        xsT = np.ascontiguousarray(xs.T).astype(np.float16)  # [D, NS]
        if has_benc:
            pad = np.zeros((P, NS), np.float16)
            pad[0, :] = 1.0
            xsT = np.concatenate([xsT, pad], axis=0)
        in_maps.append({"xT": xsT, "wencT": wencT, "wdec16": wdec16})

    res = run_bass_kernel_spmd(
        nc,
        in_maps,
        core_ids=list(range(NCORES)),
        trace=trace,
        trace_cores=trace_cores,
    )

    acts = np.concatenate(
        [res.results[c]["acts"] for c in range(NCORES)], axis=0
    ).astype(np.float32)
    recon = np.concatenate(
        [res.results[c]["recon"] for c in range(NCORES)], axis=0
    )
    mask48 = np.concatenate(
        [res.results[c]["mask48"] for c in range(NCORES)], axis=0
    )
    recon = recon + b_dec[None, :]

    # Exact boundary re-ranking. The device encode runs at FP22 (float32r)
    # and the resident raw copy is fp16, so winners whose fp32 gap is tiny
    # can swap or carry rounded values. mask48 marks each row's top-~48
    # device values; recompute those dot products exactly in fp32, rewrite
    # the row's true top-32 entries, and re-decode the few changed rows.
    rows, cols = np.nonzero(mask48)
    row_starts = np.searchsorted(rows, np.arange(N + 1))
    exact = np.einsum(
        "nd,nd->n", x_cent[rows], W_enc[cols], dtype=np.float32
    ) + b_enc[cols]
    for n in range(N):
        s, e = row_starts[n], row_starts[n + 1]
        cs = cols[s:e]
        ev = exact[s:e]
        if len(cs) < K:
            continue
        order = np.argsort(-ev, kind="stable")
        true_set = cs[order[:K]]
        sel = cs[acts[n, cs] != 0]
        changed = not (
            len(sel) == K and np.array_equal(np.sort(true_set), np.sort(sel))
        )
        acts[n, sel] = 0.0
        acts[n, true_set] = ev[order[:K]]
        if changed:
            recon[n] = acts[n, true_set] @ W_dec[true_set] + b_dec

    diff = recon.astype(np.float32) - x
    loss = np.float32(np.mean(np.sum(diff * diff, axis=-1, dtype=np.float32)))
    return (loss, recon, acts), res


def kernel(**inputs):
    out, _ = run(inputs, trace=False)
    return out


# revision 22
# speedup vs baseline: 3.3947x; 1.0204x over previous
"""TopK SAE (encode -> top-32 mask -> decode) on 8 trn2 NeuronCores.

Data-parallel over the batch dim N: each core handles N/8 = 512 rows.
W_enc is pre-transposed on the host (the PE contracts over the partition
dim, so the encode needs W_enc^T with D on partitions); W_dec is host-cast
to fp16 for the decode (products accumulate in fp32 in PSUM).

Per-core device pipeline (single pass over 4 row-tiles of 128):
  encode:   raw = x_cent @ W_enc^T via float32r (FP22) matmuls, full PE
            rate, one stream of W_enc^T from HBM
  stage1:   per 512-wide f-tile: PSUM -> fp32 staging chunk, nc.vector.max
            top-8 -> 256 exact candidates/row; chunk cast to a fp16
            resident raw (the only full-row copy SBUF can hold)
  stage2:   6x (max8 + match_replace) over the candidates -> exact
            thresholds: rank-32 (acts mask) and rank-48 (host audit mask)
  mask:     acts16 = (raw16 >= t32) * raw16 fused in place; mask48 =
            (raw16 >= t48) u8 shipped out for the host-side exact audit
  decode:   PE-transpose acts16 128x128 blocks, copy back via ScalarE,
            accumulate recon = acts @ W_dec over 128 f-chunks
Host: shard/transpose/cast prep, gather, exact re-ranking of each row's
top-48 boundary (fixes FP22/fp16 winner swaps on a few dozen rows), loss.
"""

import os
import sys

for _p in ("/opt/trn_rl_repo", "/opt/pypackages"):
    if _p not in sys.path:
        sys.path.insert(0, _p)

import numpy as np

import concourse.bacc as bacc
import concourse.mybir as mybir
from concourse.bass import ds, ts
from concourse.masks import make_identity
from concourse.tile import TileContext
from concourse.bass_utils import run_bass_kernel_spmd

# Problem shape (hardcoded per contract)
N, D, F, K = 4096, 512, 16384, 32
NCORES = 8
NS = N // NCORES          # rows per core = 512
P = 128
NT = NS // P              # 4 row-tiles per core
FT = 512                  # encode f-tile width (= stage1 chunk width)
NFT = F // FT             # 32
FG = 2                    # f-tiles per W-load group
NCAND = NFT * 8           # 256 candidates per row
FC = 128                  # decode f-chunk (transpose block)
NFC = F // FC             # 128
DECG = 8                  # decode f-chunks per group (one PSUM bank)
MCH = 2048                # mask/acts chunk width
NEG = -3.0e38

f32 = mybir.dt.float32
f32r = mybir.dt.float32r
f16 = mybir.dt.float16
u8 = mybir.dt.uint8


def build(dca: int):
    """Build the per-core Bass program. dca = number of 128-deep contraction
    chunks (4 normally, 5 when b_enc is folded in as an extra chunk)."""
    nc = bacc.Bacc("TRN2", target_bir_lowering=False)

    xT = nc.dram_tensor("xT", [dca * P, NS], f16, kind="ExternalInput")
    wencT = nc.dram_tensor("wencT", [dca * P, F], f16, kind="ExternalInput")
    wdec16 = nc.dram_tensor("wdec16", [F, D], f16, kind="ExternalInput")
    acts_d = nc.dram_tensor("acts", [NS, F], f16, kind="ExternalOutput")
    recon_d = nc.dram_tensor("recon", [NS, D], f32, kind="ExternalOutput")
    mask48_d = nc.dram_tensor("mask48", [NS, F], u8, kind="ExternalOutput")

    xT_r = xT.rearrange("(dc p) n -> p dc n", p=P)
    wencT_r = wencT.rearrange("(dc p) f -> p dc f", p=P)
    wdec_r = wdec16.rearrange("(c p) d -> p c d", p=P)

    with TileContext(nc) as tc:
        with (
            tc.tile_pool(name="persist", bufs=1) as persist,
            tc.tile_pool(name="wt", bufs=2) as wtpool,
            tc.tile_pool(name="raw", bufs=NT) as rawpool,
            tc.tile_pool(name="stage", bufs=2) as stagepool,
            tc.tile_pool(name="cand", bufs=NT) as candpool,
            tc.tile_pool(name="work", bufs=1) as workpool,
            tc.tile_pool(name="m8", bufs=NT) as m8pool,
            tc.tile_pool(name="at", bufs=2) as atpool,
            tc.tile_pool(name="wd", bufs=2) as wdpool,
            tc.tile_pool(name="rc", bufs=1) as rcpool,
            tc.tile_pool(name="m48", bufs=2) as m48pool,
            tc.tile_pool(name="encps", bufs=4, space="PSUM") as encps,
            tc.tile_pool(name="trps", bufs=2, space="PSUM") as trps,
            tc.tile_pool(name="decps", bufs=2, space="PSUM") as decps,
        ):
            ident = persist.tile([P, P], f16)
            make_identity(nc, ident)

            xt_sb = persist.tile([P, dca, NS], f16)
            nc.sync.dma_start(xt_sb[:], xT_r[:])

            raws = []
            cands = []
            for nt in range(NT):
                raws.append(rawpool.tile([P, F], f16, tag="raw", name=f"raw{nt}"))
                cands.append(
                    candpool.tile([P, NCAND], f32, tag="cand", name=f"cand{nt}")
                )

            # ---- encode + stage1 candidates (per half of row-tiles) ----
            def encode_half(half, bg=None):
                nts = [half * 2, half * 2 + 1]
                for fg in range(NFT // FG):
                    if bg is not None:
                        for _ in range(2):
                            op = next(bg, None)
                            if op is not None:
                                op()
                    wt = wtpool.tile([P, dca, FG * FT], f16, tag="wt")
                    for dc in range(dca):
                        nc.sync.dma_start(
                            wt[:, dc, :],
                            wencT_r[:, dc, ds(fg * FG * FT, FG * FT)],
                        )
                    for nt in nts:
                        pss = [
                            encps.tile(
                                [P, FT], f32, tag="encps", name=f"encps{i}"
                            )
                            for i in range(FG)
                        ]
                        for dc in range(dca):
                            for ft in range(FG):
                                nc.tensor.matmul(
                                    pss[ft],
                                    lhsT=xt_sb[:, dc, ds(nt * P, P)],
                                    rhs=wt[:, dc, ds(ft * FT, FT)],
                                    start=(dc == 0),
                                    stop=(dc == dca - 1),
                                )
                        for ft in range(FG):
                            fa = fg * FG + ft
                            st = stagepool.tile([P, FT], f32, tag="stage")
                            nc.scalar.copy(st[:], pss[ft])
                            nc.vector.max(
                                out=cands[nt][:, ds(fa * 8, 8)], in_=st[:]
                            )
                            if ft % 2 == 0:
                                nc.scalar.copy(
                                    raws[nt][:, ds(fa * FT, FT)], pss[ft]
                                )
                            else:
                                nc.vector.tensor_copy(
                                    raws[nt][:, ds(fa * FT, FT)], st[:]
                                )

            # ---- stage2: exact thresholds from candidates ----
            # 6 max8 rounds: rank K=32 -> acts threshold, rank 48 -> slack
            # threshold whose u8 mask lets the host re-rank the boundary.
            t_aps = [None] * NT
            t48_aps = [None] * NT

            def stage2_nt(nt):
                work = workpool.tile([P, NCAND], f32, tag="work")
                nc.vector.tensor_copy(work[:], cands[nt][:])
                niter = 6
                m8 = m8pool.tile([P, niter * 8], f32, tag="m8", name=f"m8_{nt}")
                for j in range(niter):
                    nc.vector.max(out=m8[:, ds(j * 8, 8)], in_=work[:])
                    if j < niter - 1:
                        nc.vector.match_replace(
                            out=work[:],
                            in_to_replace=m8[:, ds(j * 8, 8)],
                            in_values=work[:],
                            imm_value=NEG,
                        )
                t_aps[nt] = m8[:, ds(K - 1, 1)]
                t48_aps[nt] = m8[:, ds(niter * 8 - 1, 1)]

            # ---- mask + decode, interleaved in two halves so the DVE
            # mask work of half 2 overlaps the PE decode of half 1 ----
            def mask_chunk(nt, ch):
                sl = ds(ch * MCH, MCH)
                m48 = m48pool.tile([P, MCH // 2], u8, tag="m48")
                for h in range(2):
                    slh = ds(ch * MCH + h * (MCH // 2), MCH // 2)
                    nc.vector.tensor_scalar(
                        m48[:],
                        raws[nt][:, slh],
                        t48_aps[nt],
                        None,
                        op0=mybir.AluOpType.is_ge,
                    )
                    nc.sync.dma_start(mask48_d[ds(nt * P, P), slh], m48[:])
                nc.vector.scalar_tensor_tensor(
                    out=raws[nt][:, sl],
                    in0=raws[nt][:, sl],
                    scalar=t_aps[nt],
                    in1=raws[nt][:, sl],
                    op0=mybir.AluOpType.is_ge,
                    op1=mybir.AluOpType.mult,
                )
                nc.sync.dma_start(acts_d[ds(nt * P, P), sl], raws[nt][:, sl])

            def mask_ops(nts):
                for nt in nts:
                    for ch in range(F // MCH):
                        yield (lambda nt=nt, ch=ch: mask_chunk(nt, ch))

            def decode_half(half, bg=None):
                nts = [half * 2, half * 2 + 1]
                dec_ps = {
                    nt: decps.tile([P, D], f32, tag="decps", name=f"decps{nt}")
                    for nt in nts
                }
                for g in range(NFC // DECG):
                    if bg is not None:
                        for _ in range(2):
                            op = next(bg, None)
                            if op is not None:
                                op()
                    wd = wdpool.tile([P, DECG, D], f16, tag="wd")
                    nc.sync.dma_start(wd[:], wdec_r[:, ds(g * DECG, DECG), :])
                    for nt in nts:
                        tr = trps.tile([P, DECG, P], f16, tag="trps")
                        for j in range(DECG):
                            nc.tensor.transpose(
                                tr[:, j, :],
                                raws[nt][:, ds((g * DECG + j) * P, P)],
                                ident[:],
                            )
                        at = atpool.tile([P, DECG, P], f16, tag="at")
                        nc.scalar.copy(at[:], tr[:])
                        for j in range(DECG):
                            nc.tensor.matmul(
                                dec_ps[nt],
                                lhsT=at[:, j, :],
                                rhs=wd[:, j, :],
                                start=(g == 0 and j == 0),
                                stop=(g == NFC // DECG - 1 and j == DECG - 1),
                            )
                for nt in nts:
                    rc = rcpool.tile([P, D], f32, tag="rc")
                    nc.scalar.copy(rc[:], dec_ps[nt])
                    nc.sync.dma_start(recon_d[ds(nt * P, P), :], rc[:])

            encode_half(0)
            stage2_nt(0)
            stage2_nt(1)
            encode_half(1, bg=mask_ops([0, 1]))
            stage2_nt(2)
            stage2_nt(3)
            decode_half(0, bg=mask_ops([2, 3]))
            decode_half(1)

    nc.compile()
    return nc


_cache = {}


def _get_nc(dca: int):
    if dca not in _cache:
        _cache[dca] = build(dca)
    return _cache[dca]


def run(inputs, trace=False, trace_cores=None):
    x = np.asarray(inputs["x"], dtype=np.float32)
    W_enc = np.asarray(inputs["W_enc"], dtype=np.float32)
    W_dec = np.asarray(inputs["W_dec"], dtype=np.float32)
    b_enc = np.asarray(inputs["b_enc"], dtype=np.float32)
    b_dec = np.asarray(inputs["b_dec"], dtype=np.float32)
    k = int(np.asarray(inputs["num_winners"]))
    assert k == K, f"kernel specialized for K={K}, got {k}"
    assert x.shape == (N, D) and W_enc.shape == (F, D)

    x_cent = x - b_dec[None, :]

    has_benc = bool(np.any(b_enc))
    dca = D // P + (1 if has_benc else 0)

    # host-side weight prep (layout for the PE): W_enc^T with D on
    # partitions; optional extra contraction chunk folds b_enc in via an
    # all-ones row of x.
    wencT = np.ascontiguousarray(W_enc.T).astype(np.float16)  # [D, F]
    if has_benc:
        pad = np.zeros((P, F), np.float16)
        pad[0, :] = b_enc.astype(np.float16)
        wencT = np.concatenate([wencT, pad], axis=0)
    wdec16 = W_dec.astype(np.float16)              # [F, D]

    nc = _get_nc(dca)

    in_maps = []
    for c in range(NCORES):
        xs = x_cent[c * NS : (c + 1) * NS]          # [NS, D]
        xsT = np.ascontiguousarray(xs.T).astype(np.float16)  # [D, NS]
        if has_benc:
            pad = np.zeros((P, NS), np.float16)
            pad[0, :] = 1.0
            xsT = np.concatenate([xsT, pad], axis=0)
        in_maps.append({"xT": xsT, "wencT": wencT, "wdec16": wdec16})

    res = run_bass_kernel_spmd(
        nc,
        in_maps,
        core_ids=list(range(NCORES)),
        trace=trace,
        trace_cores=trace_cores,
    )

    acts = np.concatenate(
        [res.results[c]["acts"] for c in range(NCORES)], axis=0
    ).astype(np.float32)
    recon = np.concatenate(
        [res.results[c]["recon"] for c in range(NCORES)], axis=0
    )
    mask48 = np.concatenate(
        [res.results[c]["mask48"] for c in range(NCORES)], axis=0
    )
    recon = recon + b_dec[None, :]

    # Exact boundary re-ranking. The device encode runs at FP22 (float32r)
    # and the resident raw copy is fp16, so winners whose fp32 gap is tiny
    # can swap or carry rounded values. mask48 marks each row's top-~48
    # device values; recompute those dot products exactly in fp32, rewrite
    # the row's true top-32 entries, and re-decode the few changed rows.
    rows, cols = np.nonzero(mask48)
    row_starts = np.searchsorted(rows, np.arange(N + 1))
    exact = np.einsum(
        "nd,nd->n", x_cent[rows], W_enc[cols], dtype=np.float32
    ) + b_enc[cols]
    for n in range(N):
        s, e = row_starts[n], row_starts[n + 1]
        cs = cols[s:e]
        ev = exact[s:e]
        if len(cs) < K:
            continue
        order = np.argsort(-ev, kind="stable")
        true_set = cs[order[:K]]
        sel = cs[acts[n, cs] != 0]
        changed = not (
            len(sel) == K and np.array_equal(np.sort(true_set), np.sort(sel))
        )
        acts[n, sel] = 0.0
        acts[n, true_set] = ev[order[:K]]
        if changed:
            recon[n] = acts[n, true_set] @ W_dec[true_set] + b_dec

    diff = recon.astype(np.float32) - x
    loss = np.float32(np.mean(np.sum(diff * diff, axis=-1, dtype=np.float32)))
    return (loss, recon, acts), res


def kernel(**inputs):
    out, _ = run(inputs, trace=False)
    return out


# revision 23
# speedup vs baseline: 3.6895x; 1.0868x over previous
"""TopK SAE (encode -> top-32 mask -> decode) on 8 trn2 NeuronCores.

Data-parallel over the batch dim N: each core handles N/8 = 512 rows.
W_enc is pre-transposed on the host (the PE contracts over the partition
dim, so the encode needs W_enc^T with D on partitions); W_dec is host-cast
to fp16 for the decode (products accumulate in fp32 in PSUM).

Per-core device pipeline (single pass over 4 row-tiles of 128):
  encode:   raw = x_cent @ W_enc^T via float32r (FP22) matmuls, full PE
            rate, one stream of W_enc^T from HBM
  stage1:   per 512-wide f-tile: PSUM -> fp32 staging chunk, nc.vector.max
            top-8 -> 256 exact candidates/row; chunk cast to a fp16
            resident raw (the only full-row copy SBUF can hold)
  stage2:   6x (max8 + match_replace) over the candidates -> exact
            thresholds: rank-32 (acts mask) and rank-48 (host audit mask)
  mask:     acts16 = (raw16 >= t32) * raw16 fused in place; mask48 =
            (raw16 >= t48) u8 shipped out for the host-side exact audit
  decode:   PE-transpose acts16 128x128 blocks, copy back via ScalarE,
            accumulate recon = acts @ W_dec over 128 f-chunks
Host: shard/transpose/cast prep, gather, exact re-ranking of each row's
top-48 boundary (fixes FP22/fp16 winner swaps on a few dozen rows), loss.
"""

import os
import sys

for _p in ("/opt/trn_rl_repo", "/opt/pypackages"):
    if _p not in sys.path:
        sys.path.insert(0, _p)

import numpy as np

import concourse.bacc as bacc
import concourse.mybir as mybir
from concourse.bass import ds, ts
from concourse.masks import make_identity
from concourse.tile import TileContext
from concourse.bass_utils import run_bass_kernel_spmd

# Problem shape (hardcoded per contract)
N, D, F, K = 4096, 512, 16384, 32
NCORES = 8
NS = N // NCORES          # rows per core = 512
P = 128
NT = NS // P              # 4 row-tiles per core
FT = 512                  # encode f-tile width (= stage1 chunk width)
NFT = F // FT             # 32
FG = 2                    # f-tiles per W-load group
NCAND = NFT * 8           # 256 candidates per row
FC = 128                  # decode f-chunk (transpose block)
NFC = F // FC             # 128
DECG = 8                  # decode f-chunks per group (one PSUM bank)
MCH = 2048                # mask/acts chunk width
NEG = -3.0e38

f32 = mybir.dt.float32
f32r = mybir.dt.float32r
f16 = mybir.dt.float16
u8 = mybir.dt.uint8


def build(dca: int):
    """Build the per-core Bass program. dca = number of 128-deep contraction
    chunks (4 normally, 5 when b_enc is folded in as an extra chunk)."""
    nc = bacc.Bacc("TRN2", target_bir_lowering=False)

    xT = nc.dram_tensor("xT", [dca * P, NS], f16, kind="ExternalInput")
    wencT = nc.dram_tensor("wencT", [dca * P, F], f16, kind="ExternalInput")
    wdec16 = nc.dram_tensor("wdec16", [F, D], f16, kind="ExternalInput")
    acts_d = nc.dram_tensor("acts", [NS, F], f16, kind="ExternalOutput")
    recon_d = nc.dram_tensor("recon", [NS, D], f32, kind="ExternalOutput")
    mask48_d = nc.dram_tensor("mask48", [NS, F], u8, kind="ExternalOutput")

    xT_r = xT.rearrange("(dc p) n -> p dc n", p=P)
    wencT_r = wencT.rearrange("(dc p) f -> p dc f", p=P)
    wdec_r = wdec16.rearrange("(c p) d -> p c d", p=P)

    with TileContext(nc) as tc:
        with (
            tc.tile_pool(name="persist", bufs=1) as persist,
            tc.tile_pool(name="wt", bufs=2) as wtpool,
            tc.tile_pool(name="raw", bufs=NT) as rawpool,
            tc.tile_pool(name="stage", bufs=3) as stagepool,
            tc.tile_pool(name="cand", bufs=NT) as candpool,
            tc.tile_pool(name="work", bufs=1) as workpool,
            tc.tile_pool(name="m8", bufs=NT) as m8pool,
            tc.tile_pool(name="at", bufs=3) as atpool,
            tc.tile_pool(name="wd", bufs=3) as wdpool,
            tc.tile_pool(name="rc", bufs=1) as rcpool,
            tc.tile_pool(name="m48", bufs=2) as m48pool,
            tc.tile_pool(name="encps", bufs=4, space="PSUM") as encps,
            tc.tile_pool(name="trps", bufs=2, space="PSUM") as trps,
            tc.tile_pool(name="decps", bufs=2, space="PSUM") as decps,
        ):
            ident = persist.tile([P, P], f16)
            make_identity(nc, ident)

            xt_sb = persist.tile([P, dca, NS], f16)
            nc.sync.dma_start(xt_sb[:], xT_r[:])

            raws = []
            cands = []
            for nt in range(NT):
                raws.append(rawpool.tile([P, F], f16, tag="raw", name=f"raw{nt}"))
                cands.append(
                    candpool.tile([P, NCAND], f32, tag="cand", name=f"cand{nt}")
                )

            # ---- encode + stage1 candidates (per half of row-tiles) ----
            def encode_half(half, bg=None):
                nts = [half * 2, half * 2 + 1]
                for fg in range(NFT // FG):
                    if bg is not None:
                        for _ in range(2):
                            op = next(bg, None)
                            if op is not None:
                                op()
                    wt = wtpool.tile([P, dca, FG * FT], f16, tag="wt")
                    for dc in range(dca):
                        nc.sync.dma_start(
                            wt[:, dc, :],
                            wencT_r[:, dc, ds(fg * FG * FT, FG * FT)],
                        )
                    for nt in nts:
                        pss = [
                            encps.tile(
                                [P, FT], f32, tag="encps", name=f"encps{i}"
                            )
                            for i in range(FG)
                        ]
                        for dc in range(dca):
                            for ft in range(FG):
                                nc.tensor.matmul(
                                    pss[ft],
                                    lhsT=xt_sb[:, dc, ds(nt * P, P)],
                                    rhs=wt[:, dc, ds(ft * FT, FT)],
                                    start=(dc == 0),
                                    stop=(dc == dca - 1),
                                )
                        for ft in range(FG):
                            fa = fg * FG + ft
                            st = stagepool.tile([P, FT], f32, tag="stage")
                            nc.scalar.copy(st[:], pss[ft])
                            nc.vector.max(
                                out=cands[nt][:, ds(fa * 8, 8)], in_=st[:]
                            )
                            if ft % 2 == 0:
                                nc.scalar.copy(
                                    raws[nt][:, ds(fa * FT, FT)], pss[ft]
                                )
                            else:
                                nc.vector.tensor_copy(
                                    raws[nt][:, ds(fa * FT, FT)], st[:]
                                )

            # ---- stage2: exact thresholds from candidates ----
            # 6 max8 rounds: rank K=32 -> acts threshold, rank 48 -> slack
            # threshold whose u8 mask lets the host re-rank the boundary.
            t_aps = [None] * NT
            t48_aps = [None] * NT

            def stage2_nt(nt):
                work = workpool.tile([P, NCAND], f32, tag="work")
                nc.vector.tensor_copy(work[:], cands[nt][:])
                niter = 6
                m8 = m8pool.tile([P, niter * 8], f32, tag="m8", name=f"m8_{nt}")
                for j in range(niter):
                    nc.vector.max(out=m8[:, ds(j * 8, 8)], in_=work[:])
                    if j < niter - 1:
                        nc.vector.match_replace(
                            out=work[:],
                            in_to_replace=m8[:, ds(j * 8, 8)],
                            in_values=work[:],
                            imm_value=NEG,
                        )
                t_aps[nt] = m8[:, ds(K - 1, 1)]
                t48_aps[nt] = m8[:, ds(niter * 8 - 1, 1)]

            # ---- mask + decode, interleaved in two halves so the DVE
            # mask work of half 2 overlaps the PE decode of half 1 ----
            def mask_chunk(nt, ch):
                sl = ds(ch * MCH, MCH)
                m48 = m48pool.tile([P, MCH // 2], u8, tag="m48")
                for h in range(2):
                    slh = ds(ch * MCH + h * (MCH // 2), MCH // 2)
                    nc.vector.tensor_scalar(
                        m48[:],
                        raws[nt][:, slh],
                        t48_aps[nt],
                        None,
                        op0=mybir.AluOpType.is_ge,
                    )
                    nc.sync.dma_start(mask48_d[ds(nt * P, P), slh], m48[:])
                nc.vector.scalar_tensor_tensor(
                    out=raws[nt][:, sl],
                    in0=raws[nt][:, sl],
                    scalar=t_aps[nt],
                    in1=raws[nt][:, sl],
                    op0=mybir.AluOpType.is_ge,
                    op1=mybir.AluOpType.mult,
                )
                nc.sync.dma_start(acts_d[ds(nt * P, P), sl], raws[nt][:, sl])

            def mask_ops(nts):
                for nt in nts:
                    for ch in range(F // MCH):
                        yield (lambda nt=nt, ch=ch: mask_chunk(nt, ch))

            def decode_half(half, bg=None):
                nts = [half * 2, half * 2 + 1]
                dec_ps = {
                    nt: decps.tile([P, D], f32, tag="decps", name=f"decps{nt}")
                    for nt in nts
                }
                for g in range(NFC // DECG):
                    if bg is not None:
                        for _ in range(2):
                            op = next(bg, None)
                            if op is not None:
                                op()
                    wd = wdpool.tile([P, DECG, D], f16, tag="wd")
                    nc.sync.dma_start(wd[:], wdec_r[:, ds(g * DECG, DECG), :])
                    for nt in nts:
                        tr = trps.tile([P, DECG, P], f16, tag="trps")
                        for j in range(DECG):
                            nc.tensor.transpose(
                                tr[:, j, :],
                                raws[nt][:, ds((g * DECG + j) * P, P)],
                                ident[:],
                            )
                        at = atpool.tile([P, DECG, P], f16, tag="at")
                        nc.scalar.copy(at[:], tr[:])
                        for j in range(DECG):
                            nc.tensor.matmul(
                                dec_ps[nt],
                                lhsT=at[:, j, :],
                                rhs=wd[:, j, :],
                                start=(g == 0 and j == 0),
                                stop=(g == NFC // DECG - 1 and j == DECG - 1),
                            )
                for nt in nts:
                    rc = rcpool.tile([P, D], f32, tag="rc")
                    nc.scalar.copy(rc[:], dec_ps[nt])
                    nc.sync.dma_start(recon_d[ds(nt * P, P), :], rc[:])

            encode_half(0)
            stage2_nt(0)
            stage2_nt(1)
            encode_half(1, bg=mask_ops([0, 1]))
            stage2_nt(2)
            stage2_nt(3)
            decode_half(0, bg=mask_ops([2, 3]))
            decode_half(1)

    nc.compile()
    return nc


_cache = {}


def _get_nc(dca: int):
    if dca not in _cache:
        _cache[dca] = build(dca)
    return _cache[dca]


def run(inputs, trace=False, trace_cores=None):
    x = np.asarray(inputs["x"], dtype=np.float32)
    W_enc = np.asarray(inputs["W_enc"], dtype=np.float32)
    W_dec = np.asarray(inputs["W_dec"], dtype=np.float32)
    b_enc = np.asarray(inputs["b_enc"], dtype=np.float32)
    b_dec = np.asarray(inputs["b_dec"], dtype=np.float32)
    k = int(np.asarray(inputs["num_winners"]))
    assert k == K, f"kernel specialized for K={K}, got {k}"
    assert x.shape == (N, D) and W_enc.shape == (F, D)

    x_cent = x - b_dec[None, :]

    has_benc = bool(np.any(b_enc))
    dca = D // P + (1 if has_benc else 0)

    # host-side weight prep (layout for the PE): W_enc^T with D on
    # partitions; optional extra contraction chunk folds b_enc in via an
    # all-ones row of x.
    wencT = np.ascontiguousarray(W_enc.T).astype(np.float16)  # [D, F]
    if has_benc:
        pad = np.zeros((P, F), np.float16)
        pad[0, :] = b_enc.astype(np.float16)
        wencT = np.concatenate([wencT, pad], axis=0)
    wdec16 = W_dec.astype(np.float16)              # [F, D]

    nc = _get_nc(dca)

    in_maps = []
    for c in range(NCORES):
        xs = x_cent[c * NS : (c + 1) * NS]          # [NS, D]
        xsT = np.ascontiguousarray(xs.T).astype(np.float16)  # [D, NS]
        if has_benc:
            pad = np.zeros((P, NS), np.float16)
            pad[0, :] = 1.0
            xsT = np.concatenate([xsT, pad], axis=0)
        in_maps.append({"xT": xsT, "wencT": wencT, "wdec16": wdec16})

    res = run_bass_kernel_spmd(
        nc,
        in_maps,
        core_ids=list(range(NCORES)),
        trace=trace,
        trace_cores=trace_cores,
    )

    acts = np.concatenate(
        [res.results[c]["acts"] for c in range(NCORES)], axis=0
    ).astype(np.float32)
    recon = np.concatenate(
        [res.results[c]["recon"] for c in range(NCORES)], axis=0
    )
    mask48 = np.concatenate(
        [res.results[c]["mask48"] for c in range(NCORES)], axis=0
    )
    recon = recon + b_dec[None, :]

    # Exact boundary re-ranking. The device encode runs at FP22 (float32r)
    # and the resident raw copy is fp16, so winners whose fp32 gap is tiny
    # can swap or carry rounded values. mask48 marks each row's top-~48
    # device values; recompute those dot products exactly in fp32, rewrite
    # the row's true top-32 entries, and re-decode the few changed rows.
    rows, cols = np.nonzero(mask48)
    row_starts = np.searchsorted(rows, np.arange(N + 1))
    exact = np.einsum(
        "nd,nd->n", x_cent[rows], W_enc[cols], dtype=np.float32
    ) + b_enc[cols]
    for n in range(N):
        s, e = row_starts[n], row_starts[n + 1]
        cs = cols[s:e]
        ev = exact[s:e]
        if len(cs) < K:
            continue
        order = np.argsort(-ev, kind="stable")
        true_set = cs[order[:K]]
        sel = cs[acts[n, cs] != 0]
        changed = not (
            len(sel) == K and np.array_equal(np.sort(true_set), np.sort(sel))
        )
        acts[n, sel] = 0.0
        acts[n, true_set] = ev[order[:K]]
        if changed:
            recon[n] = acts[n, true_set] @ W_dec[true_set] + b_dec

    diff = recon.astype(np.float32) - x
    loss = np.float32(np.mean(np.sum(diff * diff, axis=-1, dtype=np.float32)))
    return (loss, recon, acts), res


def kernel(**inputs):
    out, _ = run(inputs, trace=False)
    return out


# revision 24
# speedup vs baseline: 3.7551x; 1.0178x over previous
"""TopK SAE (encode -> top-32 mask -> decode) on 8 trn2 NeuronCores.

Data-parallel over the batch dim N: each core handles N/8 = 512 rows.
W_enc is pre-transposed on the host (the PE contracts over the partition
dim, so the encode needs W_enc^T with D on partitions); W_dec is host-cast
to fp16 for the decode (products accumulate in fp32 in PSUM).

Per-core device pipeline (single pass over 4 row-tiles of 128):
  encode:   raw = x_cent @ W_enc^T via float32r (FP22) matmuls, full PE
            rate, one stream of W_enc^T from HBM
  stage1:   per 512-wide f-tile: PSUM -> fp32 staging chunk, nc.vector.max
            top-8 -> 256 exact candidates/row; chunk cast to a fp16
            resident raw (the only full-row copy SBUF can hold)
  stage2:   6x (max8 + match_replace) over the candidates -> exact
            thresholds: rank-32 (acts mask) and rank-48 (host audit mask)
  mask:     acts16 = (raw16 >= t32) * raw16 fused in place; mask48 =
            (raw16 >= t48) u8 shipped out for the host-side exact audit
  decode:   PE-transpose acts16 128x128 blocks, copy back via ScalarE,
            accumulate recon = acts @ W_dec over 128 f-chunks
Host: shard/transpose/cast prep, gather, exact re-ranking of each row's
top-48 boundary (fixes FP22/fp16 winner swaps on a few dozen rows), loss.
"""

import os
import sys

for _p in ("/opt/trn_rl_repo", "/opt/pypackages"):
    if _p not in sys.path:
        sys.path.insert(0, _p)

import numpy as np

import concourse.bacc as bacc
import concourse.mybir as mybir
from concourse.bass import ds, ts
from concourse.masks import make_identity
from concourse.tile import TileContext
from concourse.bass_utils import run_bass_kernel_spmd

# Problem shape (hardcoded per contract)
N, D, F, K = 4096, 512, 16384, 32
NCORES = 8
NS = N // NCORES          # rows per core = 512
P = 128
NT = NS // P              # 4 row-tiles per core
FT = 512                  # encode f-tile width (= stage1 chunk width)
NFT = F // FT             # 32
FG = 2                    # f-tiles per W-load group
NCAND = NFT * 8           # 256 candidates per row
FC = 128                  # decode f-chunk (transpose block)
NFC = F // FC             # 128
DECG = 8                  # decode f-chunks per group (one PSUM bank)
MCH = 2048                # mask/acts chunk width
NEG = -3.0e38

f32 = mybir.dt.float32
f32r = mybir.dt.float32r
f16 = mybir.dt.float16
u8 = mybir.dt.uint8


def build(dca: int):
    """Build the per-core Bass program. dca = number of 128-deep contraction
    chunks (4 normally, 5 when b_enc is folded in as an extra chunk)."""
    nc = bacc.Bacc("TRN2", target_bir_lowering=False)

    xT = nc.dram_tensor("xT", [dca * P, NS], f16, kind="ExternalInput")
    wencT = nc.dram_tensor("wencT", [dca * P, F], f16, kind="ExternalInput")
    wdec16 = nc.dram_tensor("wdec16", [F, D], f16, kind="ExternalInput")
    acts_d = nc.dram_tensor("acts", [NS, F], f16, kind="ExternalOutput")
    recon_d = nc.dram_tensor("recon", [NS, D], f32, kind="ExternalOutput")
    mask48_d = nc.dram_tensor("mask48", [NS, F], u8, kind="ExternalOutput")

    xT_r = xT.rearrange("(dc p) n -> p dc n", p=P)
    wencT_r = wencT.rearrange("(dc p) f -> p dc f", p=P)
    wdec_r = wdec16.rearrange("(c p) d -> p c d", p=P)

    with TileContext(nc) as tc:
        with (
            tc.tile_pool(name="persist", bufs=1) as persist,
            tc.tile_pool(name="wt", bufs=3) as wtpool,
            tc.tile_pool(name="raw", bufs=NT) as rawpool,
            tc.tile_pool(name="stage", bufs=3) as stagepool,
            tc.tile_pool(name="cand", bufs=NT) as candpool,
            tc.tile_pool(name="work", bufs=1) as workpool,
            tc.tile_pool(name="m8", bufs=NT) as m8pool,
            tc.tile_pool(name="at", bufs=3) as atpool,
            tc.tile_pool(name="wd", bufs=3) as wdpool,
            tc.tile_pool(name="rc", bufs=1) as rcpool,
            tc.tile_pool(name="m48", bufs=2) as m48pool,
            tc.tile_pool(name="encps", bufs=4, space="PSUM") as encps,
            tc.tile_pool(name="trps", bufs=2, space="PSUM") as trps,
            tc.tile_pool(name="decps", bufs=2, space="PSUM") as decps,
        ):
            ident = persist.tile([P, P], f16)
            make_identity(nc, ident)

            xt_sb = persist.tile([P, dca, NS], f16)
            nc.sync.dma_start(xt_sb[:], xT_r[:])

            raws = []
            cands = []
            for nt in range(NT):
                raws.append(rawpool.tile([P, F], f16, tag="raw", name=f"raw{nt}"))
                cands.append(
                    candpool.tile([P, NCAND], f32, tag="cand", name=f"cand{nt}")
                )

            # ---- encode + stage1 candidates (per half of row-tiles) ----
            def encode_half(half, bg=None):
                nts = [half * 2, half * 2 + 1]
                for fg in range(NFT // FG):
                    if bg is not None:
                        for _ in range(2):
                            op = next(bg, None)
                            if op is not None:
                                op()
                    wt = wtpool.tile([P, dca, FG * FT], f16, tag="wt")
                    for dc in range(dca):
                        nc.sync.dma_start(
                            wt[:, dc, :],
                            wencT_r[:, dc, ds(fg * FG * FT, FG * FT)],
                        )
                    for nt in nts:
                        pss = [
                            encps.tile(
                                [P, FT], f32, tag="encps", name=f"encps{i}"
                            )
                            for i in range(FG)
                        ]
                        for dc in range(dca):
                            for ft in range(FG):
                                nc.tensor.matmul(
                                    pss[ft],
                                    lhsT=xt_sb[:, dc, ds(nt * P, P)],
                                    rhs=wt[:, dc, ds(ft * FT, FT)],
                                    start=(dc == 0),
                                    stop=(dc == dca - 1),
                                )
                        for ft in range(FG):
                            fa = fg * FG + ft
                            st = stagepool.tile([P, FT], f32, tag="stage")
                            nc.scalar.copy(st[:], pss[ft])
                            nc.vector.max(
                                out=cands[nt][:, ds(fa * 8, 8)], in_=st[:]
                            )
                            if ft % 2 == 0:
                                nc.scalar.copy(
                                    raws[nt][:, ds(fa * FT, FT)], pss[ft]
                                )
                            else:
                                nc.vector.tensor_copy(
                                    raws[nt][:, ds(fa * FT, FT)], st[:]
                                )

            # ---- stage2: exact thresholds from candidates ----
            # 6 max8 rounds: rank K=32 -> acts threshold, rank 48 -> slack
            # threshold whose u8 mask lets the host re-rank the boundary.
            t_aps = [None] * NT
            t48_aps = [None] * NT

            def stage2_nt(nt):
                work = workpool.tile([P, NCAND], f32, tag="work")
                nc.vector.tensor_copy(work[:], cands[nt][:])
                niter = 6
                m8 = m8pool.tile([P, niter * 8], f32, tag="m8", name=f"m8_{nt}")
                for j in range(niter):
                    nc.vector.max(out=m8[:, ds(j * 8, 8)], in_=work[:])
                    if j < niter - 1:
                        nc.vector.match_replace(
                            out=work[:],
                            in_to_replace=m8[:, ds(j * 8, 8)],
                            in_values=work[:],
                            imm_value=NEG,
                        )
                t_aps[nt] = m8[:, ds(K - 1, 1)]
                t48_aps[nt] = m8[:, ds(niter * 8 - 1, 1)]

            # ---- mask + decode, interleaved in two halves so the DVE
            # mask work of half 2 overlaps the PE decode of half 1 ----
            def mask_chunk(nt, ch):
                sl = ds(ch * MCH, MCH)
                m48 = m48pool.tile([P, MCH // 2], u8, tag="m48")
                for h in range(2):
                    slh = ds(ch * MCH + h * (MCH // 2), MCH // 2)
                    nc.vector.tensor_scalar(
                        m48[:],
                        raws[nt][:, slh],
                        t48_aps[nt],
                        None,
                        op0=mybir.AluOpType.is_ge,
                    )
                    nc.sync.dma_start(mask48_d[ds(nt * P, P), slh], m48[:])
                nc.vector.scalar_tensor_tensor(
                    out=raws[nt][:, sl],
                    in0=raws[nt][:, sl],
                    scalar=t_aps[nt],
                    in1=raws[nt][:, sl],
                    op0=mybir.AluOpType.is_ge,
                    op1=mybir.AluOpType.mult,
                )
                nc.sync.dma_start(acts_d[ds(nt * P, P), sl], raws[nt][:, sl])

            def mask_ops(nts):
                for nt in nts:
                    for ch in range(F // MCH):
                        yield (lambda nt=nt, ch=ch: mask_chunk(nt, ch))

            def decode_half(half, bg=None):
                nts = [half * 2, half * 2 + 1]
                dec_ps = {
                    nt: decps.tile([P, D], f32, tag="decps", name=f"decps{nt}")
                    for nt in nts
                }
                for g in range(NFC // DECG):
                    if bg is not None:
                        for _ in range(2):
                            op = next(bg, None)
                            if op is not None:
                                op()
                    wd = wdpool.tile([P, DECG, D], f16, tag="wd")
                    nc.sync.dma_start(wd[:], wdec_r[:, ds(g * DECG, DECG), :])
                    for nt in nts:
                        tr = trps.tile([P, DECG, P], f16, tag="trps")
                        for j in range(DECG):
                            nc.tensor.transpose(
                                tr[:, j, :],
                                raws[nt][:, ds((g * DECG + j) * P, P)],
                                ident[:],
                            )
                        at = atpool.tile([P, DECG, P], f16, tag="at")
                        nc.scalar.copy(at[:], tr[:])
                        for j in range(DECG):
                            nc.tensor.matmul(
                                dec_ps[nt],
                                lhsT=at[:, j, :],
                                rhs=wd[:, j, :],
                                start=(g == 0 and j == 0),
                                stop=(g == NFC // DECG - 1 and j == DECG - 1),
                            )
                for nt in nts:
                    rc = rcpool.tile([P, D], f32, tag="rc")
                    nc.scalar.copy(rc[:], dec_ps[nt])
                    nc.sync.dma_start(recon_d[ds(nt * P, P), :], rc[:])

            encode_half(0)
            stage2_nt(0)
            stage2_nt(1)
            encode_half(1, bg=mask_ops([0, 1]))
            stage2_nt(2)
            stage2_nt(3)
            decode_half(0, bg=mask_ops([2, 3]))
            decode_half(1)

    nc.compile()
    return nc


_cache = {}


def _get_nc(dca: int):
    if dca not in _cache:
        _cache[dca] = build(dca)
    return _cache[dca]


def run(inputs, trace=False, trace_cores=None):
    x = np.asarray(inputs["x"], dtype=np.float32)
    W_enc = np.asarray(inputs["W_enc"], dtype=np.float32)
    W_dec = np.asarray(inputs["W_dec"], dtype=np.float32)
    b_enc = np.asarray(inputs["b_enc"], dtype=np.float32)
    b_dec = np.asarray(inputs["b_dec"], dtype=np.float32)
    k = int(np.asarray(inputs["num_winners"]))
    assert k == K, f"kernel specialized for K={K}, got {k}"
    assert x.shape == (N, D) and W_enc.shape == (F, D)

    x_cent = x - b_dec[None, :]

    has_benc = bool(np.any(b_enc))
    dca = D // P + (1 if has_benc else 0)

    # host-side weight prep (layout for the PE): W_enc^T with D on
    # partitions; optional extra contraction chunk folds b_enc in via an
    # all-ones row of x.
    wencT = np.ascontiguousarray(W_enc.T).astype(np.float16)  # [D, F]
    if has_benc:
        pad = np.zeros((P, F), np.float16)
        pad[0, :] = b_enc.astype(np.float16)
        wencT = np.concatenate([wencT, pad], axis=0)
    wdec16 = W_dec.astype(np.float16)              # [F, D]

    nc = _get_nc(dca)

    in_maps = []
    for c in range(NCORES):
        xs = x_cent[c * NS : (c + 1) * NS]          # [NS, D]
        xsT = np.ascontiguousarray(xs.T).astype(np.float16)  # [D, NS]
        if has_benc:
            pad = np.zeros((P, NS), np.float16)
            pad[0, :] = 1.0
            xsT = np.concatenate([xsT, pad], axis=0)
        in_maps.append({"xT": xsT, "wencT": wencT, "wdec16": wdec16})

    res = run_bass_kernel_spmd(
        nc,
        in_maps,
        core_ids=list(range(NCORES)),
        trace=trace,
        trace_cores=trace_cores,
    )

    acts = np.concatenate(
        [res.results[c]["acts"] for c in range(NCORES)], axis=0
    ).astype(np.float32)
    recon = np.concatenate(
        [res.results[c]["recon"] for c in range(NCORES)], axis=0
    )
    mask48 = np.concatenate(
        [res.results[c]["mask48"] for c in range(NCORES)], axis=0
    )
    recon = recon + b_dec[None, :]

    # Exact boundary re-ranking. The device encode runs at FP22 (float32r)
    # and the resident raw copy is fp16, so winners whose fp32 gap is tiny
    # can swap or carry rounded values. mask48 marks each row's top-~48
    # device values; recompute those dot products exactly in fp32, rewrite
    # the row's true top-32 entries, and re-decode the few changed rows.
    rows, cols = np.nonzero(mask48)
    row_starts = np.searchsorted(rows, np.arange(N + 1))
    exact = np.einsum(
        "nd,nd->n", x_cent[rows], W_enc[cols], dtype=np.float32
    ) + b_enc[cols]
    for n in range(N):
        s, e = row_starts[n], row_starts[n + 1]
        cs = cols[s:e]
        ev = exact[s:e]
        if len(cs) < K:
            continue
        order = np.argsort(-ev, kind="stable")
        true_set = cs[order[:K]]
        sel = cs[acts[n, cs] != 0]
        changed = not (
            len(sel) == K and np.array_equal(np.sort(true_set), np.sort(sel))
        )
        acts[n, sel] = 0.0
        acts[n, true_set] = ev[order[:K]]
        if changed:
            recon[n] = acts[n, true_set] @ W_dec[true_set] + b_dec

    diff = recon.astype(np.float32) - x
    loss = np.float32(np.mean(np.sum(diff * diff, axis=-1, dtype=np.float32)))
    return (loss, recon, acts), res


def kernel(**inputs):
    out, _ = run(inputs, trace=False)
    return out
